# revision 22
# baseline (speedup 1.0000x reference)
"""GroupedQueryAttention forward on 8 Trainium2 NeuronCores (Bass/Tile).

Sharding (per spec hint): data-parallel over batch (B=2) x tensor-parallel
over KV-head groups (4 groups of 2 KV heads + their 8 query heads each).
Core c -> (batch b = c // 4, group g = c % 4).

Each core computes, for its batch element and its 8 query heads:
  qT/kT projections in transposed layout (lhsT = W, rhs = xT), V natural via
  on-chip PE transpose of vT; causal softmax without max-subtraction (scores
  are ~N(0,1) after the 1/sqrt(hd) scale, exp cannot overflow); the softmax
  denominator is produced by the same matmul as attn@V via a ones-column
  appended to V. o_proj is row-parallel: each core emits a full [N, D] fp32
  partial, and the host sums the 4 partials per batch element.

v2 structure (vs the v1 baseline):
  - per-chunk interleave: kv proj, then per 512-token chunk
    qproj(ci) -> attention(ci) -> [qproj(ci+1) overlap] -> o_proj(ci)
  - per (pair, mt): both kv-heads' score matmuls land in one 2-bank PSUM
    tile -> a single batched Exp; causal-diagonal tiles trim the dead
    query range out of scores/exp/attnV; mask multiply shrinks to the
    128x128 triangular block.
  - softmax denominators: reciprocal_approx_fast + GpSimd partition
    broadcast (replaces fp32 PE broadcast matmuls of v1).
All device compute is bf16 with fp32 PSUM accumulation.
"""

import numpy as np

import concourse.bass as bass  # noqa: F401  (import keeps engine registry warm)
import concourse.mybir as mybir
import concourse.tile as tile
from concourse import bacc, bass_utils

# Problem shape (hardcoded per contract).
B, N, D = 2, 2048, 2048
NUM_HEADS = 32
NUM_KV_HEADS = 8
HD = 64
G = NUM_HEADS // NUM_KV_HEADS
N_CORES = 8
NT = D // 128                # 16 contraction tiles
CH = 512
NCHUNK = N // CH             # 4

_CACHE = {}


def _build():
    nc = bacc.Bacc("TRN2", target_bir_lowering=False, debug=False,
                   num_devices=N_CORES)
    f32, bf16 = mybir.dt.float32, mybir.dt.bfloat16
    Exp = mybir.ActivationFunctionType.Exp

    xT = nc.dram_tensor("xT", [D, N], bf16, kind="ExternalInput")
    wq = nc.dram_tensor("wq", [D, 512], bf16, kind="ExternalInput")
    wk = nc.dram_tensor("wk", [D, 128], bf16, kind="ExternalInput")
    wv = nc.dram_tensor("wv", [D, 128], bf16, kind="ExternalInput")
    wo = nc.dram_tensor("wo", [512, D], bf16, kind="ExternalInput")
    msk = nc.dram_tensor("msk", [128, 128], bf16, kind="ExternalInput")
    ngt = nc.dram_tensor("ngt", [128, 128], bf16, kind="ExternalInput")
    iden = nc.dram_tensor("iden", [128, 128], bf16, kind="ExternalInput")
    part = nc.dram_tensor("part", [N, D], bf16, kind="ExternalOutput")

    with tile.TileContext(nc) as tc:
        with (
            tc.tile_pool(name="sb", bufs=1) as sb,
            tc.tile_pool(name="ps", bufs=2, space="PSUM") as ps,
        ):
            # ---- input DMAs, ordered so compute can start early ----------
            wk_t = sb.tile([128, NT * 128], bf16, tag="wk")
            nc.scalar.dma_start(
                wk_t[:].rearrange("p (t o) -> p t o", t=NT),
                wk.ap().rearrange("(t p) o -> p t o", p=128))
            xt = sb.tile([128, NT * N], bf16, tag="xt")
            xr = xt[:].rearrange("p (t n) -> p t n", t=NT)
            xsrc = xT.ap().rearrange("(t p) n -> p t n", p=128)
            wq_t = sb.tile([128, NT * 512], bf16, tag="wq")
            wo_t = sb.tile([128, 4 * D], bf16, tag="wo")
            wv_t = sb.tile([128, NT * 128], bf16, tag="wv")
            for tb in range(4):       # chunk 0 in 4 sub-loads of 4 t-tiles
                nc.sync.dma_start(xr[:, 4 * tb:4 * (tb + 1), 0:CH],
                                  xsrc[:, 4 * tb:4 * (tb + 1), 0:CH])
            nc.scalar.dma_start(
                wv_t[:].rearrange("p (t o) -> p t o", t=NT),
                wv.ap().rearrange("(t p) o -> p t o", p=128))
            id_t = sb.tile([128, 128], bf16, tag="iden")
            nc.sync.dma_start(id_t[:], iden.ap()[:])
            msk_t = sb.tile([128, 128], bf16, tag="msk")
            nc.sync.dma_start(msk_t[:], msk.ap()[:])
            negtri_t = sb.tile([128, 128], bf16, tag="ngt")
            nc.sync.dma_start(negtri_t[:], ngt.ap()[:])
            for j in range(1, NCHUNK):
                nc.sync.dma_start(xr[:, :, j * CH:(j + 1) * CH],
                                  xsrc[:, :, j * CH:(j + 1) * CH])
                if j == 1:
                    nc.scalar.dma_start(
                        wq_t[:].rearrange("p (t o) -> p t o", t=NT),
                        wq.ap().rearrange("(t p) o -> p t o", p=128))
            nc.scalar.dma_start(
                wo_t[:].rearrange("p (t o) -> p t o", t=4),
                wo.ap().rearrange("(t p) o -> p t o", p=128))

            # ---- k/v projections + V transpose, chunk by chunk -----------
            kt2 = sb.tile([128, N], bf16, tag="kt2")
            v3 = sb.tile([128, 16 * 130], bf16, tag="v3")
            nc.vector.memset(v3[:], 1.0)
            for j in range(NCHUNK):
                psk = ps.tile([128, CH], f32, tag="ss")
                for t in range(NT):
                    nc.tensor.matmul(
                        psk[:], wk_t[:, t * 128:(t + 1) * 128],
                        xt[:, t * N + j * CH: t * N + (j + 1) * CH],
                        start=(t == 0), stop=(t == NT - 1))
                nc.vector.tensor_copy(kt2[:, j * CH:(j + 1) * CH], psk[:])
                psv = ps.tile([128, CH], f32, tag="ss")
                for t in range(NT):
                    nc.tensor.matmul(
                        psv[:], wv_t[:, t * 128:(t + 1) * 128],
                        xt[:, t * N + j * CH: t * N + (j + 1) * CH],
                        start=(t == 0), stop=(t == NT - 1))
                vt_s = sb.tile([128, CH], bf16, tag="vt", bufs=2)
                nc.vector.tensor_copy(vt_s[:], psv[:])
                for s4 in range(4):
                    mt = 4 * j + s4
                    pst = ps.tile([128, 128], bf16, tag="pa")
                    nc.tensor.transpose(pst[:], vt_s[:, s4 * 128:(s4 + 1) * 128],
                                        id_t[:])
                    nc.vector.tensor_copy(v3[:, mt * 130: mt * 130 + 64],
                                          pst[:, 0:64])
                    nc.vector.tensor_copy(v3[:, mt * 130 + 65: mt * 130 + 129],
                                          pst[:, 64:128])

            # ---- q projection for one chunk ------------------------------
            qt = sb.tile([128, 4 * N], bf16, tag="qt")   # [p, (pair a, n)]

            def qproj_pair(ci, a):
                psq = ps.tile([128, CH], f32, tag="ss")
                for t in range(NT):
                    nc.tensor.matmul(
                        psq[:],
                        wq_t[:, t * 512 + a * 128: t * 512 + (a + 1) * 128],
                        xt[:, t * N + ci * CH: t * N + (ci + 1) * CH],
                        start=(t == 0), stop=(t == NT - 1))
                nc.vector.tensor_copy(
                    qt[:, a * N + ci * CH: a * N + (ci + 1) * CH], psq[:])

            def qproj(ci):
                for a in range(4):
                    qproj_pair(ci, a)

            # fill queue: deferred PE work (qproj / o_proj units) dripped
            # into the attention mt loops to cover ACT-bound exp stalls
            from collections import deque
            fills = deque()

            def fill_one():
                if fills:
                    fills.popleft()()

            # ---- attention for one chunk ---------------------------------
            an_tiles = {}

            def attention(ci):
                mt_hi = 4 * (ci + 1)
                LAG = 4
                for a in range(4):
                    pa_t = ps.tile([128, 2 * CH], f32, tag="pa")
                    pts = {}
                    los = {}

                    def score_stage(mt):
                        s = mt - 4 * ci
                        lo = 128 * s if s > 0 else 0
                        los[mt] = lo
                        ss_t = ps.tile([128, 2 * CH], f32, tag="ss")
                        nc.tensor.matmul(
                            ss_t[:, lo:CH], kt2[0:64, mt * 128:(mt + 1) * 128],
                            qt[0:64, a * N + ci * CH + lo: a * N + (ci + 1) * CH],
                            start=True, stop=(s < 0), skip_group_check=True)
                        nc.tensor.matmul(
                            ss_t[:, CH + lo:2 * CH],
                            kt2[64:128, mt * 128:(mt + 1) * 128],
                            qt[64:128, a * N + ci * CH + lo: a * N + (ci + 1) * CH],
                            start=True, stop=(s < 0), skip_group_check=True)
                        if s >= 0:
                            nc.tensor.matmul(
                                ss_t[:, lo:lo + 128], negtri_t[:], id_t[:],
                                start=False, stop=True, skip_group_check=True)
                            nc.tensor.matmul(
                                ss_t[:, CH + lo:CH + lo + 128], negtri_t[:],
                                id_t[:], start=False, stop=True,
                                skip_group_check=True)
                        pt = sb.tile([128, 2 * CH], bf16, tag="pt", bufs=8)
                        if lo == 0:
                            nc.scalar.activation(pt[:], ss_t[:], Exp, scale=0.125)
                        else:
                            ss3 = ss_t[:].rearrange("p (h q) -> p h q", h=2)
                            pt3 = pt[:].rearrange("p (h q) -> p h q", h=2)
                            nc.scalar.activation(pt3[:, :, lo:CH],
                                                 ss3[:, :, lo:CH], Exp,
                                                 scale=0.125)
                        pts[mt] = pt

                    def av_stage(mt):
                        lo = los[mt]
                        pt = pts.pop(mt)
                        nc.tensor.matmul(
                            pa_t[0:65, lo:CH], v3[:, mt * 130: mt * 130 + 65],
                            pt[:, lo:CH], start=(mt == 0),
                            stop=(mt == mt_hi - 1), skip_group_check=True)
                        nc.tensor.matmul(
                            pa_t[0:65, CH + lo:2 * CH],
                            v3[:, mt * 130 + 65: mt * 130 + 130],
                            pt[:, CH + lo:2 * CH], start=(mt == 0),
                            stop=(mt == mt_hi - 1), skip_group_check=True)

                    for g in range(0, mt_hi + LAG, 2):
                        for m in (g, g + 1):
                            if m < mt_hi:
                                score_stage(m)
                        for m in (g, g + 1):
                            if LAG <= m < mt_hi + LAG:
                                av_stage(m - LAG)
                        if g + 2 < mt_hi + LAG:
                            fill_one()
                    # softmax denominators -> reciprocal -> broadcast
                    dsum = sb.tile([1, 2 * CH], f32, tag="dsum", bufs=4)
                    nc.vector.tensor_copy(dsum[:], pa_t[64:65, :])
                    rsum = sb.tile([1, 2 * CH], f32, tag="rsum", bufs=4)
                    nc.vector.reciprocal_approx_fast(rsum[:], dsum[:])
                    rb = sb.tile([128, 2 * CH], f32, tag="rb", bufs=4)
                    nc.gpsimd.partition_broadcast(rb[:], rsum[:])
                    an = sb.tile([128, CH], bf16, tag="an", bufs=8)
                    nc.vector.tensor_mul(an[0:64, :], pa_t[0:64, 0:CH],
                                         rb[0:64, 0:CH])
                    nc.vector.tensor_copy(an[64:128, :], pa_t[0:64, CH:2 * CH])
                    nc.vector.tensor_mul(an[64:128, :], an[64:128, :],
                                         rb[64:128, CH:2 * CH])
                    an_tiles[(ci, a)] = an

            # ---- o_proj unit: one [128q, 512d] output block --------------
            def po_unit(ci, nt, dc):
                po = ps.tile([128, CH], f32, tag="ss")
                for a in range(4):
                    nc.tensor.matmul(
                        po[:],
                        an_tiles[(ci, a)][:, nt * 128:(nt + 1) * 128],
                        wo_t[:, a * D + dc * CH: a * D + (dc + 1) * CH],
                        start=(a == 0), stop=(a == 3))
                st = sb.tile([128, CH], bf16, tag="st", bufs=4)
                nc.vector.tensor_copy(st[:], po[:])
                eng = nc.sync if (dc % 2 == 0) else nc.scalar
                eng.dma_start(
                    part.ap()[ci * CH + nt * 128: ci * CH + (nt + 1) * 128,
                              dc * CH:(dc + 1) * CH],
                    st[:])

            def push_oproj(ci):
                for nt in range(4):
                    for dc in range(4):
                        fills.append(lambda ci=ci, nt=nt, dc=dc:
                                     po_unit(ci, nt, dc))

            def push_qproj(ci):
                for a in range(4):
                    fills.append(lambda ci=ci, a=a: qproj_pair(ci, a))

            # ---- interleaved schedule ------------------------------------
            qproj(0)
            push_qproj(1)
            attention(0)
            for ci in range(1, NCHUNK):
                while fills:          # safety: qproj(ci) must precede
                    fill_one()        # attention(ci) emission
                if ci + 1 < NCHUNK:
                    push_qproj(ci + 1)
                push_oproj(ci - 1)
                attention(ci)
            while fills:
                fill_one()
            for nt in range(4):
                for dc in range(4):
                    po_unit(NCHUNK - 1, nt, dc)
    nc.compile()
    return nc


def _prep_in_maps(x, Wq, Wk, Wv, Wo):
    import jax.numpy as jnp

    def to_bf16(a):
        return np.asarray(jnp.asarray(np.asarray(a), dtype=jnp.bfloat16))

    # triangular mask for the 128x128 diagonal block: keep key i <= query j
    i = np.arange(128)[:, None]
    j = np.arange(128)[None, :]
    msk = (i <= j).astype(np.float32)
    ngt = np.where(j > i, np.float32(-1e9), np.float32(0.0))
    iden = np.eye(128, dtype=np.float32)

    in_maps = []
    for c in range(N_CORES):
        b, g = c // 4, c % 4
        qh = [8 * g + a for a in range(8)]      # global q heads for this core
        # Wq columns reordered into pair chunks [head a | head a+4]
        wq_cols = []
        for a in range(4):
            wq_cols.append(np.arange(qh[a] * HD, (qh[a] + 1) * HD))
            wq_cols.append(np.arange(qh[a + 4] * HD, (qh[a + 4] + 1) * HD))
        wq_r = np.asarray(Wq)[:, np.concatenate(wq_cols)]
        wo_r = np.asarray(Wo)[np.concatenate(wq_cols), :]
        wk_s = np.asarray(Wk)[:, 2 * g * HD: (2 * g + 2) * HD]
        wv_s = np.asarray(Wv)[:, 2 * g * HD: (2 * g + 2) * HD]
        in_maps.append({
            "xT": to_bf16(np.asarray(x)[b].T),
            "wq": to_bf16(wq_r),
            "wk": to_bf16(wk_s),
            "wv": to_bf16(wv_s),
            "wo": to_bf16(wo_r),
            "msk": to_bf16(msk),
            "ngt": to_bf16(ngt),
            "iden": to_bf16(iden),
        })
    return in_maps


def kernel(x, Wq, Wk, Wv, Wo, trace=False):
    if "nc" not in _CACHE:
        _CACHE["nc"] = _build()
    nc = _CACHE["nc"]
    in_maps = _prep_in_maps(x, Wq, Wk, Wv, Wo)
    res = bass_utils.run_bass_kernel_spmd(
        nc, in_maps, core_ids=list(range(N_CORES)), trace=trace)
    _CACHE["last_result"] = res
    out = np.zeros((B, N, D), np.float32)
    for c in range(N_CORES):
        out[c // 4] += np.asarray(res.results[c]["part"], dtype=np.float32)
    return out


# revision 23
# speedup vs baseline: 1.0130x; 1.0130x over previous
"""GroupedQueryAttention forward on 8 Trainium2 NeuronCores (Bass/Tile).

Sharding (per spec hint): data-parallel over batch (B=2) x tensor-parallel
over KV-head groups (4 groups of 2 KV heads + their 8 query heads each).
Core c -> (batch b = c // 4, group g = c % 4).

Each core computes, for its batch element and its 8 query heads:
  qT/kT projections in transposed layout (lhsT = W, rhs = xT), V natural via
  on-chip PE transpose of vT; causal softmax without max-subtraction (scores
  are ~N(0,1) after the 1/sqrt(hd) scale, exp cannot overflow); the softmax
  denominator is produced by the same matmul as attn@V via a ones-column
  appended to V. o_proj is row-parallel: each core emits a full [N, D] fp32
  partial, and the host sums the 4 partials per batch element.

v2 structure (vs the v1 baseline):
  - per-chunk interleave: kv proj, then per 512-token chunk
    qproj(ci) -> attention(ci) -> [qproj(ci+1) overlap] -> o_proj(ci)
  - per (pair, mt): both kv-heads' score matmuls land in one 2-bank PSUM
    tile -> a single batched Exp; causal-diagonal tiles trim the dead
    query range out of scores/exp/attnV; mask multiply shrinks to the
    128x128 triangular block.
  - softmax denominators: reciprocal_approx_fast + GpSimd partition
    broadcast (replaces fp32 PE broadcast matmuls of v1).
All device compute is bf16 with fp32 PSUM accumulation.
"""

import numpy as np

import concourse.bass as bass  # noqa: F401  (import keeps engine registry warm)
import concourse.mybir as mybir
import concourse.tile as tile
from concourse import bacc, bass_utils

# Problem shape (hardcoded per contract).
B, N, D = 2, 2048, 2048
NUM_HEADS = 32
NUM_KV_HEADS = 8
HD = 64
G = NUM_HEADS // NUM_KV_HEADS
N_CORES = 8
NT = D // 128                # 16 contraction tiles
CH = 512
NCHUNK = N // CH             # 4

_CACHE = {}


def _build():
    nc = bacc.Bacc("TRN2", target_bir_lowering=False, debug=False,
                   num_devices=N_CORES)
    f32, bf16 = mybir.dt.float32, mybir.dt.bfloat16
    Exp = mybir.ActivationFunctionType.Exp

    xT = nc.dram_tensor("xT", [D, N], bf16, kind="ExternalInput")
    wq = nc.dram_tensor("wq", [D, 512], bf16, kind="ExternalInput")
    wk = nc.dram_tensor("wk", [D, 128], bf16, kind="ExternalInput")
    wv = nc.dram_tensor("wv", [D, 128], bf16, kind="ExternalInput")
    wo = nc.dram_tensor("wo", [512, D], bf16, kind="ExternalInput")
    msk = nc.dram_tensor("msk", [128, 128], bf16, kind="ExternalInput")
    ngt = nc.dram_tensor("ngt", [128, 128], bf16, kind="ExternalInput")
    iden = nc.dram_tensor("iden", [128, 128], bf16, kind="ExternalInput")
    part = nc.dram_tensor("part", [N, D], bf16, kind="ExternalOutput")

    with tile.TileContext(nc) as tc:
        with (
            tc.tile_pool(name="sb", bufs=1) as sb,
            tc.tile_pool(name="ps", bufs=2, space="PSUM") as ps,
        ):
            # ---- input DMAs, ordered so compute can start early ----------
            wk_t = sb.tile([128, NT * 128], bf16, tag="wk")
            nc.scalar.dma_start(
                wk_t[:].rearrange("p (t o) -> p t o", t=NT),
                wk.ap().rearrange("(t p) o -> p t o", p=128))
            xt = sb.tile([128, NT * N], bf16, tag="xt")
            xr = xt[:].rearrange("p (t n) -> p t n", t=NT)
            xsrc = xT.ap().rearrange("(t p) n -> p t n", p=128)
            wq_t = sb.tile([128, NT * 512], bf16, tag="wq")
            wo_t = sb.tile([128, 4 * D], bf16, tag="wo")
            wv_t = sb.tile([128, NT * 128], bf16, tag="wv")
            for tb in range(4):       # chunk 0 in 4 sub-loads of 4 t-tiles
                nc.sync.dma_start(xr[:, 4 * tb:4 * (tb + 1), 0:CH],
                                  xsrc[:, 4 * tb:4 * (tb + 1), 0:CH])
            nc.scalar.dma_start(
                wv_t[:].rearrange("p (t o) -> p t o", t=NT),
                wv.ap().rearrange("(t p) o -> p t o", p=128))
            id_t = sb.tile([128, 128], bf16, tag="iden")
            nc.sync.dma_start(id_t[:], iden.ap()[:])
            msk_t = sb.tile([128, 128], bf16, tag="msk")
            nc.sync.dma_start(msk_t[:], msk.ap()[:])
            negtri_t = sb.tile([128, 128], bf16, tag="ngt")
            nc.sync.dma_start(negtri_t[:], ngt.ap()[:])
            for j in range(1, NCHUNK):
                nc.sync.dma_start(xr[:, :, j * CH:(j + 1) * CH],
                                  xsrc[:, :, j * CH:(j + 1) * CH])
                if j == 1:
                    nc.scalar.dma_start(
                        wq_t[:].rearrange("p (t o) -> p t o", t=NT),
                        wq.ap().rearrange("(t p) o -> p t o", p=128))
            nc.scalar.dma_start(
                wo_t[:].rearrange("p (t o) -> p t o", t=4),
                wo.ap().rearrange("(t p) o -> p t o", p=128))

            # ---- k/v projections + V transpose, chunk by chunk -----------
            kt2 = sb.tile([128, N], bf16, tag="kt2")
            v3 = sb.tile([128, 16 * 130], bf16, tag="v3")
            nc.vector.memset(v3[:], 1.0)
            for j in range(NCHUNK):
                psk = ps.tile([128, CH], f32, tag="ss")
                for t in range(NT):
                    nc.tensor.matmul(
                        psk[:], wk_t[:, t * 128:(t + 1) * 128],
                        xt[:, t * N + j * CH: t * N + (j + 1) * CH],
                        start=(t == 0), stop=(t == NT - 1))
                nc.vector.tensor_copy(kt2[:, j * CH:(j + 1) * CH], psk[:])
                psv = ps.tile([128, CH], f32, tag="ss")
                for t in range(NT):
                    nc.tensor.matmul(
                        psv[:], wv_t[:, t * 128:(t + 1) * 128],
                        xt[:, t * N + j * CH: t * N + (j + 1) * CH],
                        start=(t == 0), stop=(t == NT - 1))
                vt_s = sb.tile([128, CH], bf16, tag="vt", bufs=2)
                nc.vector.tensor_copy(vt_s[:], psv[:])
                for s4 in range(4):
                    mt = 4 * j + s4
                    pst = ps.tile([128, 128], bf16, tag="pa")
                    nc.tensor.transpose(pst[:], vt_s[:, s4 * 128:(s4 + 1) * 128],
                                        id_t[:])
                    nc.vector.tensor_copy(v3[:, mt * 130: mt * 130 + 64],
                                          pst[:, 0:64])
                    nc.vector.tensor_copy(v3[:, mt * 130 + 65: mt * 130 + 129],
                                          pst[:, 64:128])

            # ---- q projection for one chunk ------------------------------
            qt = sb.tile([128, 4 * N], bf16, tag="qt")   # [p, (pair a, n)]

            def qproj_pair(ci, a):
                psq = ps.tile([128, CH], f32, tag="ss")
                for t in range(NT):
                    nc.tensor.matmul(
                        psq[:],
                        wq_t[:, t * 512 + a * 128: t * 512 + (a + 1) * 128],
                        xt[:, t * N + ci * CH: t * N + (ci + 1) * CH],
                        start=(t == 0), stop=(t == NT - 1))
                nc.vector.tensor_copy(
                    qt[:, a * N + ci * CH: a * N + (ci + 1) * CH], psq[:])

            def qproj(ci):
                for a in range(4):
                    qproj_pair(ci, a)

            # fill queue: deferred PE work (qproj / o_proj units) dripped
            # into the attention mt loops to cover ACT-bound exp stalls
            from collections import deque
            fills = deque()

            def fill_one():
                if fills:
                    fills.popleft()()

            # ---- attention for one chunk ---------------------------------
            an_tiles = {}

            def attention(ci):
                mt_hi = 4 * (ci + 1)
                LAG = 4
                for a in range(4):
                    pa_t = ps.tile([128, 2 * CH], f32, tag="pa")
                    pts = {}
                    los = {}

                    def score_stage(mt):
                        s = mt - 4 * ci
                        lo = 128 * s if s > 0 else 0
                        los[mt] = lo
                        ss_t = ps.tile([128, 2 * CH], f32, tag="ss")
                        nc.tensor.matmul(
                            ss_t[:, lo:CH], kt2[0:64, mt * 128:(mt + 1) * 128],
                            qt[0:64, a * N + ci * CH + lo: a * N + (ci + 1) * CH],
                            start=True, stop=(s < 0), skip_group_check=True)
                        nc.tensor.matmul(
                            ss_t[:, CH + lo:2 * CH],
                            kt2[64:128, mt * 128:(mt + 1) * 128],
                            qt[64:128, a * N + ci * CH + lo: a * N + (ci + 1) * CH],
                            start=True, stop=(s < 0), skip_group_check=True)
                        if s >= 0:
                            nc.tensor.matmul(
                                ss_t[:, lo:lo + 128], negtri_t[:], id_t[:],
                                start=False, stop=True, skip_group_check=True)
                            nc.tensor.matmul(
                                ss_t[:, CH + lo:CH + lo + 128], negtri_t[:],
                                id_t[:], start=False, stop=True,
                                skip_group_check=True)
                        pt = sb.tile([128, 2 * CH], bf16, tag="pt", bufs=8)
                        if lo == 0:
                            nc.scalar.activation(pt[:], ss_t[:], Exp, scale=0.125)
                        else:
                            ss3 = ss_t[:].rearrange("p (h q) -> p h q", h=2)
                            pt3 = pt[:].rearrange("p (h q) -> p h q", h=2)
                            nc.scalar.activation(pt3[:, :, lo:CH],
                                                 ss3[:, :, lo:CH], Exp,
                                                 scale=0.125)
                        pts[mt] = pt

                    def av_stage(mt):
                        lo = los[mt]
                        pt = pts.pop(mt)
                        nc.tensor.matmul(
                            pa_t[0:65, lo:CH], v3[:, mt * 130: mt * 130 + 65],
                            pt[:, lo:CH], start=(mt == 0),
                            stop=(mt == mt_hi - 1), skip_group_check=True)
                        nc.tensor.matmul(
                            pa_t[0:65, CH + lo:2 * CH],
                            v3[:, mt * 130 + 65: mt * 130 + 130],
                            pt[:, CH + lo:2 * CH], start=(mt == 0),
                            stop=(mt == mt_hi - 1), skip_group_check=True)

                    for g in range(0, mt_hi + LAG, 2):
                        for m in (g, g + 1):
                            if m < mt_hi:
                                score_stage(m)
                        for m in (g, g + 1):
                            if LAG <= m < mt_hi + LAG:
                                av_stage(m - LAG)
                        if g + 2 < mt_hi + LAG:
                            fill_one()
                    # softmax denominators -> reciprocal -> broadcast
                    dsum = sb.tile([1, 2 * CH], f32, tag="dsum", bufs=4)
                    nc.vector.tensor_copy(dsum[:], pa_t[64:65, :])
                    rsum = sb.tile([1, 2 * CH], f32, tag="rsum", bufs=4)
                    nc.vector.reciprocal_approx_fast(rsum[:], dsum[:])
                    rb = sb.tile([128, 2 * CH], f32, tag="rb", bufs=4)
                    nc.gpsimd.partition_broadcast(rb[:], rsum[:])
                    an = sb.tile([128, CH], bf16, tag="an", bufs=8)
                    nc.vector.tensor_mul(an[0:64, :], pa_t[0:64, 0:CH],
                                         rb[0:64, 0:CH])
                    nc.vector.tensor_copy(an[64:128, :], pa_t[0:64, CH:2 * CH])
                    nc.vector.tensor_mul(an[64:128, :], an[64:128, :],
                                         rb[64:128, CH:2 * CH])
                    an_tiles[(ci, a)] = an

            # ---- o_proj unit: one [128q, 512d] output block --------------
            def po_unit(ci, nt, dc):
                po = ps.tile([128, CH], f32, tag="ss")
                for a in range(4):
                    nc.tensor.matmul(
                        po[:],
                        an_tiles[(ci, a)][:, nt * 128:(nt + 1) * 128],
                        wo_t[:, a * D + dc * CH: a * D + (dc + 1) * CH],
                        start=(a == 0), stop=(a == 3))
                st = sb.tile([128, CH], bf16, tag="st", bufs=4)
                nc.vector.tensor_copy(st[:], po[:])
                nc.sync.dma_start(
                    part.ap()[ci * CH + nt * 128: ci * CH + (nt + 1) * 128,
                              dc * CH:(dc + 1) * CH],
                    st[:])

            def push_oproj(ci):
                for nt in range(4):
                    for dc in range(4):
                        fills.append(lambda ci=ci, nt=nt, dc=dc:
                                     po_unit(ci, nt, dc))

            def push_qproj(ci):
                for a in range(4):
                    fills.append(lambda ci=ci, a=a: qproj_pair(ci, a))

            # ---- interleaved schedule ------------------------------------
            qproj(0)
            push_qproj(1)
            attention(0)
            for ci in range(1, NCHUNK):
                while fills:          # safety: qproj(ci) must precede
                    fill_one()        # attention(ci) emission
                if ci + 1 < NCHUNK:
                    push_qproj(ci + 1)
                push_oproj(ci - 1)
                attention(ci)
            while fills:
                fill_one()
            for nt in range(4):
                for dc in range(4):
                    po_unit(NCHUNK - 1, nt, dc)
    nc.compile()
    return nc


def _prep_in_maps(x, Wq, Wk, Wv, Wo):
    import jax.numpy as jnp

    def to_bf16(a):
        return np.asarray(jnp.asarray(np.asarray(a), dtype=jnp.bfloat16))

    # triangular mask for the 128x128 diagonal block: keep key i <= query j
    i = np.arange(128)[:, None]
    j = np.arange(128)[None, :]
    msk = (i <= j).astype(np.float32)
    ngt = np.where(j > i, np.float32(-1e9), np.float32(0.0))
    iden = np.eye(128, dtype=np.float32)

    in_maps = []
    for c in range(N_CORES):
        b, g = c // 4, c % 4
        qh = [8 * g + a for a in range(8)]      # global q heads for this core
        # Wq columns reordered into pair chunks [head a | head a+4]
        wq_cols = []
        for a in range(4):
            wq_cols.append(np.arange(qh[a] * HD, (qh[a] + 1) * HD))
            wq_cols.append(np.arange(qh[a + 4] * HD, (qh[a + 4] + 1) * HD))
        wq_r = np.asarray(Wq)[:, np.concatenate(wq_cols)]
        wo_r = np.asarray(Wo)[np.concatenate(wq_cols), :]
        wk_s = np.asarray(Wk)[:, 2 * g * HD: (2 * g + 2) * HD]
        wv_s = np.asarray(Wv)[:, 2 * g * HD: (2 * g + 2) * HD]
        in_maps.append({
            "xT": to_bf16(np.asarray(x)[b].T),
            "wq": to_bf16(wq_r),
            "wk": to_bf16(wk_s),
            "wv": to_bf16(wv_s),
            "wo": to_bf16(wo_r),
            "msk": to_bf16(msk),
            "ngt": to_bf16(ngt),
            "iden": to_bf16(iden),
        })
    return in_maps


def kernel(x, Wq, Wk, Wv, Wo, trace=False):
    if "nc" not in _CACHE:
        _CACHE["nc"] = _build()
    nc = _CACHE["nc"]
    in_maps = _prep_in_maps(x, Wq, Wk, Wv, Wo)
    res = bass_utils.run_bass_kernel_spmd(
        nc, in_maps, core_ids=list(range(N_CORES)), trace=trace)
    _CACHE["last_result"] = res
    out = np.zeros((B, N, D), np.float32)
    for c in range(N_CORES):
        out[c // 4] += np.asarray(res.results[c]["part"], dtype=np.float32)
    return out


# revision 24
# speedup vs baseline: 1.0257x; 1.0125x over previous
"""GroupedQueryAttention forward on 8 Trainium2 NeuronCores (Bass/Tile).

Sharding (per spec hint): data-parallel over batch (B=2) x tensor-parallel
over KV-head groups (4 groups of 2 KV heads + their 8 query heads each).
Core c -> (batch b = c // 4, group g = c % 4).

Each core computes, for its batch element and its 8 query heads:
  qT/kT projections in transposed layout (lhsT = W, rhs = xT), V natural via
  on-chip PE transpose of vT; causal softmax without max-subtraction (scores
  are ~N(0,1) after the 1/sqrt(hd) scale, exp cannot overflow); the softmax
  denominator is produced by the same matmul as attn@V via a ones-column
  appended to V. o_proj is row-parallel: each core emits a full [N, D] fp32
  partial, and the host sums the 4 partials per batch element.

v2 structure (vs the v1 baseline):
  - per-chunk interleave: kv proj, then per 512-token chunk
    qproj(ci) -> attention(ci) -> [qproj(ci+1) overlap] -> o_proj(ci)
  - per (pair, mt): both kv-heads' score matmuls land in one 2-bank PSUM
    tile -> a single batched Exp; causal-diagonal tiles trim the dead
    query range out of scores/exp/attnV; mask multiply shrinks to the
    128x128 triangular block.
  - softmax denominators: reciprocal_approx_fast + GpSimd partition
    broadcast (replaces fp32 PE broadcast matmuls of v1).
All device compute is bf16 with fp32 PSUM accumulation.
"""

import numpy as np

import concourse.bass as bass  # noqa: F401  (import keeps engine registry warm)
import concourse.mybir as mybir
import concourse.tile as tile
from concourse import bacc, bass_utils

# Problem shape (hardcoded per contract).
B, N, D = 2, 2048, 2048
NUM_HEADS = 32
NUM_KV_HEADS = 8
HD = 64
G = NUM_HEADS // NUM_KV_HEADS
N_CORES = 8
NT = D // 128                # 16 contraction tiles
CH = 512
NCHUNK = N // CH             # 4

_CACHE = {}


def _build():
    nc = bacc.Bacc("TRN2", target_bir_lowering=False, debug=False,
                   num_devices=N_CORES)
    f32, bf16 = mybir.dt.float32, mybir.dt.bfloat16
    Exp = mybir.ActivationFunctionType.Exp

    xT = nc.dram_tensor("xT", [D, N], bf16, kind="ExternalInput")
    wq = nc.dram_tensor("wq", [D, 512], bf16, kind="ExternalInput")
    wk = nc.dram_tensor("wk", [D, 128], bf16, kind="ExternalInput")
    wv = nc.dram_tensor("wv", [D, 128], bf16, kind="ExternalInput")
    wo = nc.dram_tensor("wo", [512, D], bf16, kind="ExternalInput")
    msk = nc.dram_tensor("msk", [128, 128], bf16, kind="ExternalInput")
    ngt = nc.dram_tensor("ngt", [128, 128], bf16, kind="ExternalInput")
    iden = nc.dram_tensor("iden", [128, 128], bf16, kind="ExternalInput")
    part = nc.dram_tensor("part", [N, D], bf16, kind="ExternalOutput")

    with tile.TileContext(nc) as tc:
        with (
            tc.tile_pool(name="sb", bufs=1) as sb,
            tc.tile_pool(name="ps", bufs=2, space="PSUM") as ps,
        ):
            # ---- input DMAs, ordered so compute can start early ----------
            wk_t = sb.tile([128, NT * 128], bf16, tag="wk")
            nc.sync.dma_start(
                wk_t[:].rearrange("p (t o) -> p t o", t=NT),
                wk.ap().rearrange("(t p) o -> p t o", p=128))
            xt = sb.tile([128, NT * N], bf16, tag="xt")
            xr = xt[:].rearrange("p (t n) -> p t n", t=NT)
            xsrc = xT.ap().rearrange("(t p) n -> p t n", p=128)
            wq_t = sb.tile([128, NT * 512], bf16, tag="wq")
            wo_t = sb.tile([128, 4 * D], bf16, tag="wo")
            wv_t = sb.tile([128, NT * 128], bf16, tag="wv")
            for tb in range(4):       # chunk 0 in 4 sub-loads of 4 t-tiles
                nc.sync.dma_start(xr[:, 4 * tb:4 * (tb + 1), 0:CH],
                                  xsrc[:, 4 * tb:4 * (tb + 1), 0:CH])
            nc.sync.dma_start(
                wv_t[:].rearrange("p (t o) -> p t o", t=NT),
                wv.ap().rearrange("(t p) o -> p t o", p=128))
            id_t = sb.tile([128, 128], bf16, tag="iden")
            nc.sync.dma_start(id_t[:], iden.ap()[:])
            msk_t = sb.tile([128, 128], bf16, tag="msk")
            nc.sync.dma_start(msk_t[:], msk.ap()[:])
            negtri_t = sb.tile([128, 128], bf16, tag="ngt")
            nc.sync.dma_start(negtri_t[:], ngt.ap()[:])
            for j in range(1, NCHUNK):
                nc.sync.dma_start(xr[:, :, j * CH:(j + 1) * CH],
                                  xsrc[:, :, j * CH:(j + 1) * CH])
                if j == 1:
                    nc.sync.dma_start(
                        wq_t[:].rearrange("p (t o) -> p t o", t=NT),
                        wq.ap().rearrange("(t p) o -> p t o", p=128))
            nc.sync.dma_start(
                wo_t[:].rearrange("p (t o) -> p t o", t=4),
                wo.ap().rearrange("(t p) o -> p t o", p=128))

            # ---- k/v projections + V transpose, chunk by chunk -----------
            kt2 = sb.tile([128, N], bf16, tag="kt2")
            v3 = sb.tile([128, 16 * 130], bf16, tag="v3")
            nc.vector.memset(v3[:], 1.0)
            for j in range(NCHUNK):
                psk = ps.tile([128, CH], f32, tag="ss")
                for t in range(NT):
                    nc.tensor.matmul(
                        psk[:], wk_t[:, t * 128:(t + 1) * 128],
                        xt[:, t * N + j * CH: t * N + (j + 1) * CH],
                        start=(t == 0), stop=(t == NT - 1))
                nc.vector.tensor_copy(kt2[:, j * CH:(j + 1) * CH], psk[:])
                psv = ps.tile([128, CH], f32, tag="ss")
                for t in range(NT):
                    nc.tensor.matmul(
                        psv[:], wv_t[:, t * 128:(t + 1) * 128],
                        xt[:, t * N + j * CH: t * N + (j + 1) * CH],
                        start=(t == 0), stop=(t == NT - 1))
                vt_s = sb.tile([128, CH], bf16, tag="vt", bufs=2)
                nc.vector.tensor_copy(vt_s[:], psv[:])
                for s4 in range(4):
                    mt = 4 * j + s4
                    pst = ps.tile([128, 128], bf16, tag="pa")
                    nc.tensor.transpose(pst[:], vt_s[:, s4 * 128:(s4 + 1) * 128],
                                        id_t[:])
                    nc.vector.tensor_copy(v3[:, mt * 130: mt * 130 + 64],
                                          pst[:, 0:64])
                    nc.vector.tensor_copy(v3[:, mt * 130 + 65: mt * 130 + 129],
                                          pst[:, 64:128])

            # ---- q projection for one chunk ------------------------------
            qt = sb.tile([128, 4 * N], bf16, tag="qt")   # [p, (pair a, n)]

            def qproj_pair(ci, a):
                psq = ps.tile([128, CH], f32, tag="ss")
                for t in range(NT):
                    nc.tensor.matmul(
                        psq[:],
                        wq_t[:, t * 512 + a * 128: t * 512 + (a + 1) * 128],
                        xt[:, t * N + ci * CH: t * N + (ci + 1) * CH],
                        start=(t == 0), stop=(t == NT - 1))
                nc.vector.tensor_copy(
                    qt[:, a * N + ci * CH: a * N + (ci + 1) * CH], psq[:])

            def qproj(ci):
                for a in range(4):
                    qproj_pair(ci, a)

            # fill queue: deferred PE work (qproj / o_proj units) dripped
            # into the attention mt loops to cover ACT-bound exp stalls
            from collections import deque
            fills = deque()

            def fill_one():
                if fills:
                    fills.popleft()()

            # ---- attention for one chunk ---------------------------------
            an_tiles = {}

            def attention(ci):
                mt_hi = 4 * (ci + 1)
                LAG = 4
                for a in range(4):
                    pa_t = ps.tile([128, 2 * CH], f32, tag="pa")
                    pts = {}
                    los = {}

                    def score_stage(mt):
                        s = mt - 4 * ci
                        lo = 128 * s if s > 0 else 0
                        los[mt] = lo
                        ss_t = ps.tile([128, 2 * CH], f32, tag="ss")
                        nc.tensor.matmul(
                            ss_t[:, lo:CH], kt2[0:64, mt * 128:(mt + 1) * 128],
                            qt[0:64, a * N + ci * CH + lo: a * N + (ci + 1) * CH],
                            start=True, stop=(s < 0), skip_group_check=True)
                        nc.tensor.matmul(
                            ss_t[:, CH + lo:2 * CH],
                            kt2[64:128, mt * 128:(mt + 1) * 128],
                            qt[64:128, a * N + ci * CH + lo: a * N + (ci + 1) * CH],
                            start=True, stop=(s < 0), skip_group_check=True)
                        if s >= 0:
                            nc.tensor.matmul(
                                ss_t[:, lo:lo + 128], negtri_t[:], id_t[:],
                                start=False, stop=True, skip_group_check=True)
                            nc.tensor.matmul(
                                ss_t[:, CH + lo:CH + lo + 128], negtri_t[:],
                                id_t[:], start=False, stop=True,
                                skip_group_check=True)
                        pt = sb.tile([128, 2 * CH], bf16, tag="pt", bufs=8)
                        if lo == 0:
                            nc.scalar.activation(pt[:], ss_t[:], Exp, scale=0.125)
                        else:
                            ss3 = ss_t[:].rearrange("p (h q) -> p h q", h=2)
                            pt3 = pt[:].rearrange("p (h q) -> p h q", h=2)
                            nc.scalar.activation(pt3[:, :, lo:CH],
                                                 ss3[:, :, lo:CH], Exp,
                                                 scale=0.125)
                        pts[mt] = pt

                    def av_stage(mt):
                        lo = los[mt]
                        pt = pts.pop(mt)
                        nc.tensor.matmul(
                            pa_t[0:65, lo:CH], v3[:, mt * 130: mt * 130 + 65],
                            pt[:, lo:CH], start=(mt == 0),
                            stop=(mt == mt_hi - 1), skip_group_check=True)
                        nc.tensor.matmul(
                            pa_t[0:65, CH + lo:2 * CH],
                            v3[:, mt * 130 + 65: mt * 130 + 130],
                            pt[:, CH + lo:2 * CH], start=(mt == 0),
                            stop=(mt == mt_hi - 1), skip_group_check=True)

                    for g in range(0, mt_hi + LAG, 2):
                        for m in (g, g + 1):
                            if m < mt_hi:
                                score_stage(m)
                        for m in (g, g + 1):
                            if LAG <= m < mt_hi + LAG:
                                av_stage(m - LAG)
                        if g + 2 < mt_hi + LAG:
                            fill_one()
                    # softmax denominators -> reciprocal -> broadcast
                    dsum = sb.tile([1, 2 * CH], f32, tag="dsum", bufs=4)
                    nc.vector.tensor_copy(dsum[:], pa_t[64:65, :])
                    rsum = sb.tile([1, 2 * CH], f32, tag="rsum", bufs=4)
                    nc.vector.reciprocal_approx_fast(rsum[:], dsum[:])
                    rb = sb.tile([128, 2 * CH], f32, tag="rb", bufs=4)
                    nc.gpsimd.partition_broadcast(rb[:], rsum[:])
                    an = sb.tile([128, CH], bf16, tag="an", bufs=8)
                    nc.vector.tensor_mul(an[0:64, :], pa_t[0:64, 0:CH],
                                         rb[0:64, 0:CH])
                    nc.vector.tensor_copy(an[64:128, :], pa_t[0:64, CH:2 * CH])
                    nc.vector.tensor_mul(an[64:128, :], an[64:128, :],
                                         rb[64:128, CH:2 * CH])
                    an_tiles[(ci, a)] = an

            # ---- o_proj unit: one [128q, 512d] output block --------------
            def po_unit(ci, nt, dc):
                po = ps.tile([128, CH], f32, tag="ss")
                for a in range(4):
                    nc.tensor.matmul(
                        po[:],
                        an_tiles[(ci, a)][:, nt * 128:(nt + 1) * 128],
                        wo_t[:, a * D + dc * CH: a * D + (dc + 1) * CH],
                        start=(a == 0), stop=(a == 3))
                st = sb.tile([128, CH], bf16, tag="st", bufs=4)
                nc.vector.tensor_copy(st[:], po[:])
                nc.sync.dma_start(
                    part.ap()[ci * CH + nt * 128: ci * CH + (nt + 1) * 128,
                              dc * CH:(dc + 1) * CH],
                    st[:])

            def push_oproj(ci):
                for nt in range(4):
                    for dc in range(4):
                        fills.append(lambda ci=ci, nt=nt, dc=dc:
                                     po_unit(ci, nt, dc))

            def push_qproj(ci):
                for a in range(4):
                    fills.append(lambda ci=ci, a=a: qproj_pair(ci, a))

            # ---- interleaved schedule ------------------------------------
            qproj(0)
            push_qproj(1)
            attention(0)
            for ci in range(1, NCHUNK):
                while fills:          # safety: qproj(ci) must precede
                    fill_one()        # attention(ci) emission
                if ci + 1 < NCHUNK:
                    push_qproj(ci + 1)
                push_oproj(ci - 1)
                attention(ci)
            while fills:
                fill_one()
            for nt in range(4):
                for dc in range(4):
                    po_unit(NCHUNK - 1, nt, dc)
    nc.compile()
    return nc


def _prep_in_maps(x, Wq, Wk, Wv, Wo):
    import jax.numpy as jnp

    def to_bf16(a):
        return np.asarray(jnp.asarray(np.asarray(a), dtype=jnp.bfloat16))

    # triangular mask for the 128x128 diagonal block: keep key i <= query j
    i = np.arange(128)[:, None]
    j = np.arange(128)[None, :]
    msk = (i <= j).astype(np.float32)
    ngt = np.where(j > i, np.float32(-1e9), np.float32(0.0))
    iden = np.eye(128, dtype=np.float32)

    in_maps = []
    for c in range(N_CORES):
        b, g = c // 4, c % 4
        qh = [8 * g + a for a in range(8)]      # global q heads for this core
        # Wq columns reordered into pair chunks [head a | head a+4]
        wq_cols = []
        for a in range(4):
            wq_cols.append(np.arange(qh[a] * HD, (qh[a] + 1) * HD))
            wq_cols.append(np.arange(qh[a + 4] * HD, (qh[a + 4] + 1) * HD))
        wq_r = np.asarray(Wq)[:, np.concatenate(wq_cols)]
        wo_r = np.asarray(Wo)[np.concatenate(wq_cols), :]
        wk_s = np.asarray(Wk)[:, 2 * g * HD: (2 * g + 2) * HD]
        wv_s = np.asarray(Wv)[:, 2 * g * HD: (2 * g + 2) * HD]
        in_maps.append({
            "xT": to_bf16(np.asarray(x)[b].T),
            "wq": to_bf16(wq_r),
            "wk": to_bf16(wk_s),
            "wv": to_bf16(wv_s),
            "wo": to_bf16(wo_r),
            "msk": to_bf16(msk),
            "ngt": to_bf16(ngt),
            "iden": to_bf16(iden),
        })
    return in_maps


def kernel(x, Wq, Wk, Wv, Wo, trace=False):
    if "nc" not in _CACHE:
        _CACHE["nc"] = _build()
    nc = _CACHE["nc"]
    in_maps = _prep_in_maps(x, Wq, Wk, Wv, Wo)
    res = bass_utils.run_bass_kernel_spmd(
        nc, in_maps, core_ids=list(range(N_CORES)), trace=trace)
    _CACHE["last_result"] = res
    out = np.zeros((B, N, D), np.float32)
    for c in range(N_CORES):
        out[c // 4] += np.asarray(res.results[c]["part"], dtype=np.float32)
    return out
